# revision 7
# baseline (speedup 1.0000x reference)
"""Trainium2 Bass kernel for nn_DynamicConvolution.

Reference computation (per batch b, T=4096 timesteps, C=512 channels):
    h  = x @ w_in.T + b_in                    # (T, 2C)
    xg = h[:, :C] * sigmoid(h[:, C:])         # GLU -> (T, C)
    w  = softmax((xg @ w_wt.T + b_wt).reshape(T, H, K), axis=-1)
    out[c, t] = sum_k xg[t+k-3, c] * w[t, h(c), k]    # depthwise dynamic conv
    y  = (out + conv_bias) @ w_out.T + b_out

Sharding: data-parallel over batch B=8 -> one batch element per NeuronCore.
Each core runs an identical program on its slice; no collectives.

Per-core dataflow (all matmuls bf16, fp32 accumulation), fully software-
pipelined as a single loop over 32 time tiles of 128:
  - x is host pre-transposed AND pre-cast to bf16; streamed via HWDGE.
  - mm1 produces h token-major in PSUM; GLU on ACT+DVE -> xg (token-major).
  - xg is PE-transposed to xgT (C-major) for the weight-projection matmul.
  - dynamic-weight logits are computed C-major ([56, t]); exp on ACT; the
    [64, 512] exp block is DMA-XBAR-transposed to token-major, where the
    K-sum (DVE windowed reduce), reciprocal, and normalization multiply
    run on DVE.  The normalize multiply writes wsm3 [p, k, m, h] directly
    (strided output AP does the permute for free).
  - The dynamic conv is a banded matmul per (h, time-tile): out_h =
    xg_slab.T @ D, where D[t', t] = w[h, t'-t+3, t] is a 7-diagonal band.
    D is materialized with one gpsimd local_scatter per tile from a
    pre-shifted copy of the softmax weights (data_all); the per-partition
    scatter indices are host-precomputed constants.
  - Cross-tile band halo handled by DVE edge adds between PSUM tiles.
  - mm_out contracts C (conv output is C-major already) -> y, stored bf16
    and upcast on the host.
Pass-2 work (scatter/conv/mm_out) for tile j is interleaved with pass-1
work for tile m ~ j+16, keeping Tensor/ACT/DVE/GpSimd all busy.
"""

import os
import sys

import numpy as np

for _p in ("/opt/trn_rl_repo", os.path.expanduser("~/.axon_site/_ro/trn_rl_repo")):
    if os.path.isdir(_p) and _p not in sys.path:
        sys.path.insert(0, _p)

import concourse.bacc as bacc
import concourse.bass as bass
import concourse.mybir as mybir
import concourse.tile as tile
from concourse.bass_utils import run_bass_kernel_spmd

try:
    import ml_dtypes

    BF16 = np.dtype(ml_dtypes.bfloat16)
except ImportError:  # pragma: no cover
    BF16 = None

T, B, C = 4096, 8, 512
H, K = 8, 7
PAD_L = K // 2
C2 = 2 * C
HK = H * K  # 56
P = 128

F32 = mybir.dt.float32
BF = mybir.dt.bfloat16
I16 = mybir.dt.int16

# Dt tile layout: per h a 136-wide block holding the 134 band columns of one
# 128-timestep tile (columns j <-> t = t0 + j - 3).
MAIN_W = 136
DT_W = H * MAIN_W  # 1088
CW = P + 2 * PAD_L  # 134 band columns per tile


def ts(i, size):
    return slice(i * size, (i + 1) * size)


def host_scatter_idxs():
    """Scatter index table: data element (p, i, h) -> column of the Dt tile.

    data[p, i*8+h] = wsm[t0 + p + i - 3, 7h + 6 - i]; its band column is
    j = p + i (column j of block h covers output time t0 + j - 3).
    """
    p = np.arange(P)[:, None, None]
    i = np.arange(K)[None, :, None]
    h = np.arange(H)[None, None, :]
    idx = MAIN_W * h + p + i
    return np.ascontiguousarray(idx.reshape(P, K * H).astype(np.int16))


def build_nc(t_len=T, with_bias_in=False, with_bias_wt=False, with_bias_out=False,
             with_conv_bias=False, dbg=False):
    """Build the single-core Bass program (shared by all 8 cores)."""
    NT = t_len // P   # time tiles of 128
    NB = t_len // 512  # 512-blocks for pass1c

    nc = bacc.Bacc()

    x_d = nc.declare_dram_parameter("xT", [P, 4, t_len], BF, isOutput=False)
    w_inT_d = nc.declare_dram_parameter("w_inT", [P, 4, C2], BF, isOutput=False)
    w_wtT_d = nc.declare_dram_parameter("w_wtT", [P, 4, HK], BF, isOutput=False)
    w_outT_d = nc.declare_dram_parameter("w_outT", [P, 4, C], BF, isOutput=False)
    idxs_d = nc.declare_dram_parameter("idxs", [P, HK], I16, isOutput=False)
    ident16_d = nc.declare_dram_parameter("ident16", [P, P], BF, isOutput=False)
    if with_bias_in:
        b_in_d = nc.declare_dram_parameter("b_in", [C2], F32, isOutput=False)
    if with_bias_wt:
        b_wt_d = nc.declare_dram_parameter("b_wt", [HK], F32, isOutput=False)
    if with_bias_out:
        b_out_d = nc.declare_dram_parameter("b_out", [C], F32, isOutput=False)
    if with_conv_bias:
        cb4_d = nc.declare_dram_parameter("cb4", [P, 4], F32, isOutput=False)
    y_d = nc.declare_dram_parameter("y", [t_len, C], BF, isOutput=True)
    if dbg:
        xg_dbg = nc.declare_dram_parameter("xg_dbg", [P, NT, C], BF, isOutput=True)
        xgT_dbg = nc.declare_dram_parameter("xgT_dbg", [P, 4, t_len], BF, isOutput=True)
        wsm_dbg = nc.declare_dram_parameter("wsm_dbg", [P, K, NT, H], BF, isOutput=True)
        data_dbg = nc.declare_dram_parameter("data_dbg", [P, NT, HK], BF, isOutput=True)
        conv_dbg = nc.declare_dram_parameter("conv_dbg", [P, 4, t_len], BF, isOutput=True)

    with tile.TileContext(nc) as tc:
        with (
            tc.tile_pool(name="const", bufs=1) as const,
            tc.tile_pool(name="big", bufs=1) as big,
            tc.tile_pool(name="work", bufs=3) as work,
            tc.tile_pool(name="dtp", bufs=4) as dtp,
            tc.tile_pool(name="ps", bufs=2, space=bass.MemorySpace.PSUM) as ps,
        ):
            # ---- constants ----
            sb_winT = const.tile([P, 4, C2], BF)
            nc.sync.dma_start(sb_winT[:], w_inT_d[:])
            sb_wwtT = const.tile([P, 4, HK], BF)
            nc.sync.dma_start(sb_wwtT[:], w_wtT_d[:])
            sb_woutT = const.tile([P, 4, C], BF)
            nc.sync.dma_start(sb_woutT[:], w_outT_d[:])
            sb_idxs = const.tile([P, HK], I16)
            nc.sync.dma_start(sb_idxs[:], idxs_d[:])
            sb_id16 = const.tile([P, P], BF)
            nc.sync.dma_start(sb_id16[:], ident16_d[:])
            if with_bias_in:
                sb_bin = const.tile([P, C2], F32)
                nc.sync.dma_start(sb_bin[:], b_in_d[None, :].to_broadcast((P, C2)))
            if with_bias_wt:
                sb_bwt = const.tile([HK, 1], F32)
                nc.sync.dma_start(sb_bwt[:], b_wt_d[:, None])
            if with_bias_out:
                sb_bout = const.tile([P, C], F32)
                nc.sync.dma_start(sb_bout[:], b_out_d[None, :].to_broadcast((P, C)))
            if with_conv_bias:
                sb_cb4 = const.tile([P, 4], F32)
                nc.sync.dma_start(sb_cb4[:], cb4_d[:])

            # ---- persistent activations ----
            xTs = big.tile([P, 4, t_len], BF)      # [c%128, c//128, t]
            xg = big.tile([P, NT, C], BF)          # [t%128, t//128, c]
            xgT = big.tile([P, 4, t_len], BF)      # [c%128, c//128, t]
            conv = big.tile([P, 4, t_len], BF)     # [c%128, c//128, t]
            wsm3 = big.tile([P, K, NT, H], BF)     # [t%128, k, t//128, h]
            data_tmp = big.tile([P, K, NT, H], BF)
            data_all = big.tile([P, NT, HK], BF)

            nc.gpsimd.memset(data_tmp[:], 0.0)

            # ---- x streaming: first block in tile-sized pieces so mm1 can
            # start early, the rest in 512-column blocks ----
            for mi in range(min(4, NT)):
                nc.sync.dma_start(xTs[:, :, ts(mi, P)], x_d[:, :, ts(mi, P)])
            for blk in range(1, NB):
                nc.sync.dma_start(xTs[:, :, ts(blk, 512)], x_d[:, :, ts(blk, 512)])

            # ======== per-tile pass-1: mm1 -> GLU -> xg; PE transpose ========
            def mm1_glu(m):
                psa = ps.tile([P, C], F32, tag="psa", bufs=2)
                psg = ps.tile([P, C], F32, tag="psg", bufs=2)
                for q in range(4):
                    lhs = xTs[:, q, ts(m, P)]
                    nc.tensor.matmul(psa[:], lhs, sb_winT[:, q, 0:C],
                                     start=(q == 0), stop=(q == 3))
                    nc.tensor.matmul(psg[:], lhs, sb_winT[:, q, C:C2],
                                     start=(q == 0), stop=(q == 3))
                sig = work.tile([P, C], F32, tag="sig")
                if with_bias_in:
                    tmp_g = work.tile([P, C], F32, tag="tmp_g")
                    nc.vector.tensor_add(tmp_g[:], psg[:], sb_bin[:, C:C2])
                    nc.scalar.activation(sig[:], tmp_g[:],
                                         mybir.ActivationFunctionType.Sigmoid)
                    tmp_a = work.tile([P, C], F32, tag="tmp_a")
                    nc.vector.tensor_add(tmp_a[:], psa[:], sb_bin[:, 0:C])
                    nc.vector.tensor_mul(xg[:, m, :], tmp_a[:], sig[:])
                else:
                    nc.scalar.activation(sig[:], psg[:],
                                         mybir.ActivationFunctionType.Sigmoid)
                    nc.vector.tensor_mul(xg[:, m, :], psa[:], sig[:])
                # xg -> xgT via PE transpose
                pxgT = ps.tile([P, 4, P], BF, tag="mix", bufs=2)
                for q in range(4):
                    nc.tensor.transpose(pxgT[:, q, :], xg[:, m, ts(q, P)], sb_id16[:])
                nc.scalar.copy(xgT[:, :, ts(m, P)], pxgT[:])

            # ======== pass1c (per 512-block): dynamic weights + softmax ====
            # logits C-major -> exp on ACT -> DMA-XBAR transpose to
            # token-major -> DVE windowed K-sum + reciprocal + normalize.
            # The normalize multiply writes wsm3 [p, k, m, h] via a strided
            # output AP (permute for free).
            def pass1c(n):
                pwl = ps.tile([HK, 512], F32, tag="mix", bufs=2)
                for q in range(4):
                    nc.tensor.matmul(pwl[:], sb_wwtT[:, q, :],
                                     xgT[:, q, ts(n, 512)],
                                     start=(q == 0), stop=(q == 3))
                e64 = work.tile([64, 512], BF, tag="e64", bufs=2)
                if with_bias_wt:
                    nc.scalar.activation(e64[0:HK, :], pwl[:],
                                         mybir.ActivationFunctionType.Exp,
                                         bias=sb_bwt[:])
                else:
                    nc.scalar.activation(e64[0:HK, :], pwl[:],
                                         mybir.ActivationFunctionType.Exp)
                wtok = work.tile([P, 4, 64], BF, tag="wtok", bufs=2)
                nc.sync.dma_start(wtok[:], e64[:], transpose=True)
                wtok4 = wtok[:, :, 0:HK].rearrange("p m (h k) -> p m h k", k=K)
                s8 = work.tile([P, 4, H], F32, tag="s8", bufs=2)
                nc.vector.tensor_reduce(s8[:], wtok4[:, :, :, 0:K],
                                        axis=mybir.AxisListType.X,
                                        op=mybir.AluOpType.add)
                r8 = work.tile([P, 4, H], F32, tag="r8", bufs=2)
                nc.vector.reciprocal_approx_fast(r8[:], s8[:])
                w_dst = wsm3[:, :, ts(n, 4), :].transpose([0, 2, 3, 1])
                with nc.allow_low_precision(reason="softmax weights in bf16"):
                    nc.vector.tensor_mul(
                        w_dst, wtok4[:, :, :, 0:K],
                        r8[:, :, :, None].to_broadcast((P, 4, H, K)))

            # ======== band-shift staging (per group of tiles) ========
            def build_group(mlo, mhi):
                # shifted copies of wsm3 feeding the band scatter
                for i in range(K):
                    d = i - 3
                    kk = 6 - i
                    if d == 0:
                        nc.sync.dma_start(data_tmp[:, i, mlo:mhi, :],
                                          wsm3[:, kk, mlo:mhi, :])
                    elif d < 0:
                        nc.sync.dma_start(data_tmp[-d:P, i, mlo:mhi, :],
                                          wsm3[0:P + d, kk, mlo:mhi, :])
                        lo = max(mlo, 1)
                        if lo < mhi:
                            nc.sync.dma_start(data_tmp[0:-d, i, lo:mhi, :],
                                              wsm3[P + d:P, kk, lo - 1:mhi - 1, :])
                    else:
                        nc.sync.dma_start(data_tmp[0:P - d, i, mlo:mhi, :],
                                          wsm3[d:P, kk, mlo:mhi, :])
                        hi = min(mhi, NT - 1)
                        if mlo < hi:
                            nc.sync.dma_start(data_tmp[P - d:P, i, mlo:hi, :],
                                              wsm3[0:d, kk, mlo + 1:hi + 1, :])
                # permute [p, i, m, h] -> [p, m, (i, h)]
                da4 = data_all[:, mlo:mhi, :].rearrange("p m (i h) -> p m i h", h=H)
                nc.vector.tensor_copy(
                    da4, data_tmp[:, :, mlo:mhi, :].transpose([0, 2, 1, 3]))

            # ======== pass-2 per tile: scatter -> banded conv -> mm_out ====
            dt_tiles = {}

            def scatter(j):
                dt = dtp.tile([P, DT_W], BF, tag="dt")
                nc.gpsimd.local_scatter(dt[:], data_all[:, j, :], sb_idxs[:],
                                        channels=P, num_elems=DT_W, num_idxs=HK)
                dt_tiles[j] = dt

            el_prev = {}

            def conv_tile(j):
                dt = dt_tiles.pop(j)
                t0 = j * P
                for half in range(2):
                    pch = ps.tile([P, 2, 256], F32, tag="pc", bufs=2)
                    pc = pch[:, :, 0:CW]
                    for c2 in range(2):
                        ci = 2 * half + c2
                        for hp, pb in ((0, 0), (1, 64)):
                            hh = ci * 2 + hp
                            nc.tensor.matmul(
                                pc[pb:pb + 64, c2, :], xg[:, j, ts(hh, 64)],
                                dt[:, MAIN_W * hh:MAIN_W * hh + CW],
                                start=True, stop=True, skip_group_check=True)
                    csl = conv[:, 2 * half:2 * half + 2, :]
                    # body of tile j (must precede the left-edge add)
                    if with_conv_bias:
                        for c2 in range(2):
                            ci = 2 * half + c2
                            nc.vector.tensor_scalar_add(
                                csl[:, c2, t0:t0 + P], pc[:, c2, PAD_L:PAD_L + P],
                                sb_cb4[:, ci:ci + 1])
                    else:
                        nc.scalar.copy(csl[:, :, t0:t0 + P],
                                       pc[:, :, PAD_L:PAD_L + P])
                    if j > 0:
                        # left edge of tile j: slab j-1 rows feeding t0..t0+2
                        dl = csl[:, :, t0:t0 + PAD_L]
                        nc.vector.tensor_add(dl, dl, el_prev[(j - 1, half)][:])
                        # right edge of tile j-1: slab j rows feeding its tail
                        dr = csl[:, :, t0 - PAD_L:t0]
                        nc.vector.tensor_add(dr, dr, pc[:, :, 0:PAD_L])
                    if j + 1 < NT:
                        el = work.tile([P, 2, PAD_L], F32, tag=f"el{half}", bufs=2)
                        nc.vector.tensor_copy(el[:], pc[:, :, CW - PAD_L:CW])
                        el_prev[(j, half)] = el

            def mm_out(m):
                po = ps.tile([P, C], F32, tag="mix", bufs=2)
                for q in range(4):
                    nc.tensor.matmul(po[:], conv[:, q, ts(m, P)], sb_woutT[:, q, :],
                                     start=(q == 0), stop=(q == 3))
                out_t = work.tile([P, C], BF, tag="out_t")
                if with_bias_out:
                    with nc.allow_low_precision(reason="y stored bf16"):
                        nc.vector.tensor_add(out_t[:], po[:], sb_bout[:])
                else:
                    with nc.allow_low_precision(reason="y stored bf16"):
                        nc.vector.tensor_copy(out_t[:], po[:])
                nc.sync.dma_start(y_d[ts(m, P), :], out_t[:])

            # ======== the software-pipelined main loop ========
            # pass1c runs in pairs (m % 8 == 7) to halve ACT table reloads.
            # builds: group g tiles need wsm3 of tile hi (clamped), i.e. the
            # pass1c pair covering tile hi.
            def js_for(m):
                if m < 16:
                    return []
                if m < 24:
                    return [m - 16]           # 0..7
                if m < 31:
                    j0 = 8 + 2 * (m - 24)     # 8..21
                    return [j0, j0 + 1]
                if m == 31:
                    return [22]               # 23+ gated on the last build
                return list(range(23, NT))

            for m in range(NT):
                for j in js_for(m):
                    conv_tile(j)
                    if j > 0:
                        mm_out(j - 1)
                mm1_glu(m)
                if m % 8 == 7:
                    pass1c(m // 4 - 1)
                    pass1c(m // 4)
                if m == 15:
                    build_group(0, 15)
                if m == 23:
                    build_group(15, 23)
                if m == 31:
                    build_group(23, NT)
                # issue scatters one tile ahead of conv consumption, AFTER
                # the build that produces their data (program order == dep
                # order for the tile framework).  Cap the look-ahead so we
                # never have more than `dtp` bufs of unconsumed scatters.
                for j in js_for(m + 1)[:2]:
                    scatter(j)
            for j in range(23, NT):
                conv_tile(j)
                mm_out(j - 1)
                if j + 2 < NT:
                    scatter(j + 2)
            mm_out(NT - 1)

            if dbg:
                nc.sync.dma_start(xg_dbg[:], xg[:])
                nc.sync.dma_start(xgT_dbg[:], xgT[:])
                nc.sync.dma_start(wsm_dbg[:], wsm3[:])
                nc.sync.dma_start(data_dbg[:], data_all[:])
                nc.sync.dma_start(conv_dbg[:], conv[:])

    nc.compile()
    return nc


def host_inputs(x_b, w_in, b_in, w_wt, b_wt, w_out, b_out, conv_bias,
                with_bias_in, with_bias_wt, with_bias_out, with_conv_bias):
    """Per-core input map from a batch slice + shared weights."""
    def t_pack(w, width, dt_=None):
        # w: [width, C] -> [128, 4, width] with [p, q, f] = w[f, 128q+p]
        a = np.ascontiguousarray(
            w.T.reshape(4, P, width).transpose(1, 0, 2)).astype(dt_ or BF16)
        return a

    xT = np.ascontiguousarray(
        np.asarray(x_b, np.float32).T.reshape(4, P, T).transpose(1, 0, 2)
    ).astype(BF16)
    m = {
        "xT": xT,
        "w_inT": t_pack(w_in, C2),
        "w_wtT": t_pack(w_wt, HK),
        "w_outT": t_pack(w_out, C),
        "idxs": host_scatter_idxs(),
        "ident16": np.eye(P).astype(BF16),
    }
    if with_bias_in:
        m["b_in"] = np.asarray(b_in, np.float32)
    if with_bias_wt:
        m["b_wt"] = np.asarray(b_wt, np.float32)
    if with_bias_out:
        m["b_out"] = np.asarray(b_out, np.float32)
    if with_conv_bias:
        m["cb4"] = np.ascontiguousarray(
            np.asarray(conv_bias, np.float32).reshape(4, P).T)
    return m


_NC_CACHE = {}


def _get_nc(key):
    if key not in _NC_CACHE:
        _NC_CACHE[key] = build_nc(T, *key)
    return _NC_CACHE[key]


def kernel(x, w_in, b_in, w_wt, b_wt, w_out, b_out, conv_bias, _trace=False):
    x = np.asarray(x)
    flags = (bool(np.any(b_in)), bool(np.any(b_wt)), bool(np.any(b_out)),
             bool(np.any(conv_bias)))
    nc = _get_nc(flags)
    in_maps = [
        host_inputs(x[:, b, :], np.asarray(w_in), b_in, np.asarray(w_wt), b_wt,
                    np.asarray(w_out), b_out, conv_bias, *flags)
        for b in range(B)
    ]
    res = run_bass_kernel_spmd(nc, in_maps, core_ids=list(range(B)),
                               trace=_trace)
    y = np.stack([np.asarray(res.results[b]["y"]).astype(np.float32)
                  for b in range(B)], axis=1)
    if _trace:
        return y, res
    return y
